# revision 29
# baseline (speedup 1.0000x reference)
"""Causal self-attention (B=2, S=2048, D=2048, H=16, HD=128) on 8 TRN2 cores.

Sharding: core c -> batch b = c//4, heads 4*(c%4)..4*(c%4)+3 (tensor-parallel
over heads within a batch; data-parallel over batch across core groups).

Fused single-pass design, fully SBUF-resident (no DRAM bounce):
  for each s-block sb of 512:
    - Q^T/K^T (RoPE applied) and V projections for the 4 local heads, written
      straight into SBUF homes in bf16,
    - causal attention for q-block qb==sb over k-chunks 0..4*sb+3 in
      transposed-score layout (S^T[k,q]); exp on ACT, row-sums l via a
      128-wide ones matmul (output arrives pre-broadcast over partitions),
      PV produces ctx^T[hd,q],
    - output projection for qb==sb interleaved into the next s-block's
      projection groups; partial [2048,2048] summed on host across 4 cores.
All PE operands are bf16 (full PE rate, half the DMA/SBUF traffic); PSUM
accumulation stays fp32.  Weights/x arrive via few large DMAs split across
the two HWDGE issue engines (sync + scalar) so the prologue is not
DMA-issue-bound.
"""

import math
from collections import deque

import numpy as np
import ml_dtypes

import concourse.bacc as bacc
import concourse.mybir as mybir
from concourse.tile import TileContext
from concourse.bass_utils import run_bass_kernel_spmd

B, S, D = 2, 2048, 2048
H, HD = 16, 128
ROPE_THETA = 10000.0

N_CORES = 8
CORES_PER_BATCH = 4
HPC = H // (N_CORES // B)  # heads per core = 4
HL = HPC * HD              # 512 local head-dim columns
NDC = D // 128             # 16 contraction chunks
NSB = S // 512             # 4 s-blocks

F32 = mybir.dt.float32
BF16 = mybir.dt.bfloat16
AF = mybir.ActivationFunctionType
NPBF = ml_dtypes.bfloat16


def _mm(nc, out, lhsT, rhs, start, stop):
    nc.tensor.matmul(out, lhsT, rhs, start=start, stop=stop)


def _build():
    nc = bacc.Bacc("TRN2", target_bir_lowering=False, debug=False)

    # x and W arrive pre-packed host-side as [128, chunk, cols] so each DMA
    # moves 4KB-contiguous per-partition segments (vs 1KB unpacked).
    xT = nc.dram_tensor("xT", [NSB, 128, NDC, 512], BF16, kind="ExternalInput")
    wq = nc.dram_tensor("wq", [128, NDC, HL], BF16, kind="ExternalInput")
    wk = nc.dram_tensor("wk", [128, NDC, HL], BF16, kind="ExternalInput")
    wv = nc.dram_tensor("wv", [128, NDC, HL], BF16, kind="ExternalInput")
    wo = nc.dram_tensor("wo", [HL, D], BF16, kind="ExternalInput")
    cosT = nc.dram_tensor("cosT", [HD, S], BF16, kind="ExternalInput")
    sinT = nc.dram_tensor("sinT", [HD, S], BF16, kind="ExternalInput")
    pmatT = nc.dram_tensor("pmatT", [HD, HD], BF16, kind="ExternalInput")
    maskT = nc.dram_tensor("maskT", [128, 512], BF16, kind="ExternalInput")
    onesd = nc.dram_tensor("onesd", [128, 128], BF16, kind="ExternalInput")
    out = nc.dram_tensor("out", [S, D], BF16, kind="ExternalOutput")

    with TileContext(nc) as tc:
        with (
            tc.tile_pool(name="homes", bufs=1) as homes,
            tc.tile_pool(name="consts", bufs=1) as consts,
            tc.tile_pool(name="wpool", bufs=1) as wpool,
            tc.tile_pool(name="xtp", bufs=6) as xtp,
            tc.tile_pool(name="st1", bufs=3) as st1,
            tc.tile_pool(name="ptp", bufs=6) as ptp,
            tc.tile_pool(name="smp", bufs=2) as smp,
            tc.tile_pool(name="outp", bufs=2) as outp,
            tc.tile_pool(name="psA", bufs=4, space="PSUM") as psA,
            tc.tile_pool(name="psB", bufs=2, space="PSUM") as psB,
            tc.tile_pool(name="psC", bufs=2, space="PSUM") as psC,
        ):
            # persistent SBUF homes (bf16)
            qh = [[homes.tile([HD, 512], BF16, name=f"qh{h}_{sb}")
                   for sb in range(NSB)] for h in range(HPC)]
            kh = [[homes.tile([HD, 512], BF16, name=f"kh{h}_{sb}")
                   for sb in range(NSB)] for h in range(HPC)]
            vh = [homes.tile([128, HL], BF16, name=f"vh{kc}")
                  for kc in range(4 * NSB)]
            ch = [[homes.tile([HD, 512], BF16, name=f"ch{h}_{sb}")
                   for sb in range(NSB)] for h in range(HPC)]

            pmat_sb = consts.tile([HD, HD], BF16, name="pmat_sb")
            mask_sb = consts.tile([128, 512], BF16, name="mask_sb")
            ones_sb = consts.tile([128, 128], BF16, name="ones_sb")
            cos_sb = consts.tile([HD, S], BF16, name="cos_sb")
            sin_sb = consts.tile([HD, S], BF16, name="sin_sb")

            wo_sb = wpool.tile([128, HPC, D], BF16, name="wo_sb")

            # quarter loads: [128, 4, 512] tiles, 4KB contiguous per partition
            def load_w_quarter(w_d, q4, tag, eng):
                wt = wpool.tile([128, 4, HL], BF16, tag=f"{tag}{q4}", name="wt")
                eng.dma_start(out=wt[:], in_=w_d[:, q4 * 4:(q4 + 1) * 4, :])
                return wt

            def load_x_quarter(sb, q4):
                xt = xtp.tile([128, 4, 512], BF16, tag="xt", name="xt")
                nc.sync.dma_start(out=xt[:],
                                  in_=xT[sb, :, q4 * 4:(q4 + 1) * 4, :])
                return xt

            # ---------------- prologue DMAs (dual-queue) --------------------
            # sync: x quarters + tables + wv; scalar: wq/wk interleaved + wo
            xts = []
            wq_t, wk_t, wv_t = [], [], []
            for q4 in range(4):
                xts.append(load_x_quarter(0, q4))
                wq_t.append(load_w_quarter(wq, q4, "wq", nc.scalar))
                wk_t.append(load_w_quarter(wk, q4, "wk", nc.scalar))
                if q4 == 0:
                    nc.sync.dma_start(out=cos_sb[:], in_=cosT[:])
                if q4 == 1:
                    nc.sync.dma_start(out=sin_sb[:], in_=sinT[:])
                if q4 == 2:
                    nc.sync.dma_start(out=pmat_sb[:], in_=pmatT[:])
                    nc.sync.dma_start(out=mask_sb[:], in_=maskT[:])
                    nc.sync.dma_start(out=ones_sb[:], in_=onesd[:])
            for q4 in range(4):
                wv_t.append(load_w_quarter(wv, q4, "wv", nc.sync))
            nc.scalar.dma_start(out=wo_sb[:],
                                in_=wo.rearrange("(r p) c -> p r c", p=128))

            def xchunk(xts_g, dc):
                return xts_g[dc // 4][:, dc % 4, :]

            # proj finishers, lagged one group behind emission
            finishers = deque()

            def emit_finisher():
                kind, args = finishers.popleft()
                if kind == "qk":
                    ps, dst, sl = args
                    qraw = st1.tile([128, 512], BF16, tag="qraw", name="qraw")
                    nc.scalar.activation(qraw[:], ps[:], AF.Copy)
                    rot = psB.tile([128, 512], F32, tag="b", name="rot")
                    _mm(nc, rot[:], pmat_sb[:], qraw[:], start=True, stop=True)
                    acos = st1.tile([128, 512], F32, tag="acos", name="acos")
                    nc.vector.tensor_mul(acos[:], ps[:], cos_sb[:, sl])
                    rsin = st1.tile([128, 512], F32, tag="rsin", name="rsin")
                    nc.vector.tensor_mul(rsin[:], rot[:], sin_sb[:, sl])
                    nc.vector.tensor_add(dst[:], rsin[:], acos[:])
                else:
                    ps, kc = args
                    nc.scalar.activation(vh[kc][:], ps[:], AF.Copy)

            # attention bookkeeping
            lagq = deque()    # (lps, pv, pt, vtc, ncols, first, last)
            fin = deque()     # (h, sb, lps, pv)
            lpv_done = set()

            def emit_lpv(job):
                lps, pv, pt, vtc, ncols, first, last = job
                _mm(nc, lps[:, 512 - ncols:], ones_sb[:], pt[:, :ncols],
                    start=first, stop=last)
                _mm(nc, pv[:, 512 - ncols:], vtc, pt[:, :ncols],
                    start=first, stop=last)
                if last:
                    lpv_done.add(id(lps))

            def emit_finalize(job):
                h, sb, lps, pv = job
                rcps = smp.tile([128, 512], F32, tag="rcps", name="rcps")
                rcp = smp.tile([128, 512], F32, tag="rcp", name="rcp")
                nc.vector.reciprocal_approx_accurate(rcp[:], lps[:], rcps[:])
                nc.vector.tensor_mul(ch[h][sb][:], pv[:], rcp[:])

            def pop_finalize():
                if fin and id(fin[0][2]) in lpv_done:
                    emit_finalize(fin.popleft())

            outproj_q = deque()   # (qc, db) pending output-projection groups
            osb_cur = {}          # qc -> [128, D] staging tile

            def emit_outproj():
                qc, db = outproj_q.popleft()
                sb = qc // 4
                ops = psA.tile([128, 512], F32, tag="a", name="ops")
                for h in range(HPC):
                    _mm(nc, ops[:],
                        ch[h][sb][:, (qc % 4) * 128:(qc % 4 + 1) * 128],
                        wo_sb[:, h, db * 512:(db + 1) * 512],
                        start=(h == 0), stop=(h == HPC - 1))
                if db == 0:
                    osb_cur[qc] = outp.tile([128, D], BF16, tag="osb", name="osb")
                nc.scalar.activation(
                    osb_cur[qc][:, db * 512:(db + 1) * 512], ops[:], AF.Copy)
                if db == D // 512 - 1:
                    nc.sync.dma_start(
                        out=out[qc * 128:(qc + 1) * 128, :],
                        in_=osb_cur.pop(qc)[:])

            def emit_qk_group(w_t, dst_tile, h, xts_g, sl_g):
                ps = psA.tile([128, 512], F32, tag="a", name="ps")
                for dc in range(NDC):
                    _mm(nc, ps[:],
                        w_t[dc // 4][:, dc % 4, h * HD:(h + 1) * HD],
                        xchunk(xts_g, dc),
                        start=(dc == 0), stop=(dc == NDC - 1))
                finishers.append(("qk", (ps, dst_tile, sl_g)))
                if len(finishers) > 1:
                    emit_finisher()

            def emit_v_group(sc, kc_out, xts_g):
                ps = psA.tile([128, 512], F32, tag="a", name="ps")
                for dc in range(NDC):
                    _mm(nc, ps[:],
                        xchunk(xts_g, dc)[:, sc * 128:(sc + 1) * 128],
                        wv_t[dc // 4][:, dc % 4, :],
                        start=(dc == 0), stop=(dc == NDC - 1))
                finishers.append(("v", (ps, kc_out)))
                if len(finishers) > 1:
                    emit_finisher()

            def outproj_drip(n):
                for _ in range(n):
                    if outproj_q:
                        emit_outproj()

            # ------------------------- main fused loop ----------------------
            for sb in range(NSB):
                sl = slice(sb * 512, (sb + 1) * 512)
                for h in range(HPC):
                    emit_qk_group(wq_t, qh[h][sb], h, xts, sl)
                    outproj_drip(1)
                for h in range(HPC):
                    emit_qk_group(wk_t, kh[h][sb], h, xts, sl)
                    outproj_drip(2)
                for sc in range(4):
                    emit_v_group(sc, 4 * sb + sc, xts)
                    outproj_drip(2)
                while finishers:
                    emit_finisher()
                while outproj_q:
                    emit_outproj()

                # x prefetch for the next s-block (streams during attention)
                if sb < NSB - 1:
                    xts = [load_x_quarter(sb + 1, q4) for q4 in range(4)]

                # ------- attention for q-block qb == sb, all local heads ----
                nk = 4 * sb + 4
                for h in range(HPC):
                    lps = psC.tile([128, 512], F32, tag="c", name="lps")
                    pv = psB.tile([128, 512], F32, tag="b", name="pv")
                    for kc in range(nk):
                        j = kc - 4 * sb
                        ncols = 512 if j < 0 else 512 - 128 * j
                        sps = psA.tile([128, 512], F32, tag="a", name="sps")
                        _mm(nc, sps[:, :ncols],
                            kh[h][kc // 4][:, (kc % 4) * 128:(kc % 4 + 1) * 128],
                            qh[h][sb][:, 512 - ncols:],
                            start=True, stop=True)
                        pt = ptp.tile([128, 512], BF16, tag="pt", name="pt")
                        nc.scalar.activation(pt[:, :ncols], sps[:, :ncols], AF.Exp)
                        if j >= 0:
                            nc.vector.tensor_mul(pt[:, :ncols], pt[:, :ncols],
                                                 mask_sb[:, :ncols])
                        lagq.append((lps, pv, pt,
                                     vh[kc][:, h * HD:(h + 1) * HD],
                                     ncols, kc == 0, kc == nk - 1))
                        while len(lagq) > 2:
                            emit_lpv(lagq.popleft())
                        pop_finalize()
                    fin.append((h, sb, lps, pv))
                while lagq:
                    emit_lpv(lagq.popleft())
                while fin:
                    pop_finalize()
                # queue this s-block's output projection
                for qc in range(4 * sb, 4 * sb + 4):
                    for db in range(D // 512):
                        outproj_q.append((qc, db))
            while outproj_q:
                emit_outproj()

    nc.compile()
    return nc


_NC_CACHE = None


def _get_nc():
    global _NC_CACHE
    if _NC_CACHE is None:
        _NC_CACHE = _build()
    return _NC_CACHE


def _host_tables():
    # Replicate reference RoPE tables in float32 arithmetic, transposed.
    inv_freq = np.float32(1.0) / np.power(
        np.float32(ROPE_THETA), np.arange(0, HD, 2).astype(np.float32) / np.float32(HD)
    )
    pos = np.arange(S, dtype=np.float32)
    freqs = pos[:, None] * inv_freq[None, :]
    angles = np.concatenate([freqs, freqs], axis=1)  # [S, HD]
    cos_t = np.ascontiguousarray(np.cos(angles).astype(np.float32).T).astype(NPBF)
    sin_t = np.ascontiguousarray(np.sin(angles).astype(np.float32).T).astype(NPBF)
    # rotate_half as a left-multiply matrix P: (P q)[2i] = -q[2i+1], [2i+1] = q[2i].
    # matmul computes lhsT.T @ rhs, so feed P.T.
    pmat = np.zeros((HD, HD), dtype=np.float32)
    for i in range(HD // 2):
        pmat[2 * i, 2 * i + 1] = -1.0
        pmat[2 * i + 1, 2 * i] = 1.0
    pmat_t = np.ascontiguousarray(pmat.T).astype(NPBF)
    mask = (np.arange(128)[:, None] <= np.arange(512)[None, :]).astype(NPBF)
    return cos_t, sin_t, pmat_t, mask


_ONES = np.ones((128, 128), dtype=NPBF)


def kernel(x, Wq, Wk, Wv, Wo):
    x = np.asarray(x, dtype=np.float32)
    Wq = np.asarray(Wq, dtype=np.float32)
    Wk = np.asarray(Wk, dtype=np.float32)
    Wv = np.asarray(Wv, dtype=np.float32)
    Wo = np.asarray(Wo, dtype=np.float32)

    results = _run_device(x, Wq, Wk, Wv, Wo)

    out = np.empty((B, S, D), dtype=np.float32)
    for b in range(B):
        acc = np.asarray(results[b * CORES_PER_BATCH]["out"], np.float32)
        for i in range(1, CORES_PER_BATCH):
            acc = acc + np.asarray(results[b * CORES_PER_BATCH + i]["out"],
                                   np.float32)
        out[b] = acc
    return out


def _pack_w(w):
    # [D, HL] -> [128, NDC, HL]: partition p holds rows {dc*128+p}, 4KB rows
    return np.ascontiguousarray(
        w.reshape(NDC, 128, HL).transpose(1, 0, 2)).astype(NPBF)


def _pack_x(xb):
    # x[b] [S, D] -> [NSB, 128, NDC, 512]: xT chunk layout per s-block
    xT = xb.T  # [D, S]
    return np.ascontiguousarray(
        xT.reshape(NDC, 128, NSB, 512).transpose(2, 1, 0, 3)).astype(NPBF)


def _make_in_maps(x, Wq, Wk, Wv, Wo):
    cos_t, sin_t, pmat_t, mask = _host_tables()
    scale = np.float32(1.0 / math.sqrt(HD))
    wq_scaled = (Wq * scale).astype(np.float32)
    xTb = [_pack_x(x[b]) for b in range(B)]
    in_maps = []
    for c in range(N_CORES):
        b = c // CORES_PER_BATCH
        g = c % CORES_PER_BATCH
        hs = slice(g * HL, (g + 1) * HL)
        in_maps.append({
            "xT": xTb[b],
            "wq": _pack_w(wq_scaled[:, hs]),
            "wk": _pack_w(Wk[:, hs]),
            "wv": _pack_w(Wv[:, hs]),
            "wo": np.ascontiguousarray(Wo[hs, :]).astype(NPBF),
            "cosT": cos_t,
            "sinT": sin_t,
            "pmatT": pmat_t,
            "maskT": mask,
            "onesd": _ONES,
        })
    return in_maps


def _run_device(x, Wq, Wk, Wv, Wo, trace=False):
    nc = _get_nc()
    in_maps = _make_in_maps(x, Wq, Wk, Wv, Wo)
    res = run_bass_kernel_spmd(nc, in_maps, core_ids=list(range(N_CORES)), trace=trace)
    if trace:
        return res
    return res.results


def run_traced(x, Wq, Wk, Wv, Wo):
    """Run with NTFF tracing; returns (full_output, BassKernelResults)."""
    res = _run_device(np.asarray(x, np.float32), np.asarray(Wq, np.float32),
                      np.asarray(Wk, np.float32), np.asarray(Wv, np.float32),
                      np.asarray(Wo, np.float32), trace=True)
    out = np.empty((B, S, D), dtype=np.float32)
    for b in range(B):
        acc = np.asarray(res.results[b * CORES_PER_BATCH]["out"], np.float32)
        for i in range(1, CORES_PER_BATCH):
            acc = acc + np.asarray(res.results[b * CORES_PER_BATCH + i]["out"],
                                   np.float32)
        out[b] = acc
    return out, res


# revision 31
# speedup vs baseline: 1.0003x; 1.0003x over previous
"""Causal self-attention (B=2, S=2048, D=2048, H=16, HD=128) on 8 TRN2 cores.

Sharding: core c -> batch b = c//4, heads 4*(c%4)..4*(c%4)+3 (tensor-parallel
over heads within a batch; data-parallel over batch across core groups).

Fused single-pass design, fully SBUF-resident (no DRAM bounce):
  for each s-block sb of 512:
    - Q^T/K^T (RoPE applied) and V projections for the 4 local heads, written
      straight into SBUF homes in bf16,
    - causal attention for q-block qb==sb over k-chunks 0..4*sb+3 in
      transposed-score layout (S^T[k,q]); exp on ACT, row-sums l via a
      128-wide ones matmul (output arrives pre-broadcast over partitions),
      PV produces ctx^T[hd,q],
    - output projection for qb==sb interleaved into the next s-block's
      projection groups; partial [2048,2048] summed on host across 4 cores.
All PE operands are bf16 (full PE rate, half the DMA/SBUF traffic); PSUM
accumulation stays fp32.  Weights/x arrive via few large DMAs split across
the two HWDGE issue engines (sync + scalar) so the prologue is not
DMA-issue-bound.
"""

import math
from collections import deque

import numpy as np
import ml_dtypes

import concourse.bacc as bacc
import concourse.mybir as mybir
from concourse.tile import TileContext
from concourse.bass_utils import run_bass_kernel_spmd

B, S, D = 2, 2048, 2048
H, HD = 16, 128
ROPE_THETA = 10000.0

N_CORES = 8
CORES_PER_BATCH = 4
HPC = H // (N_CORES // B)  # heads per core = 4
HL = HPC * HD              # 512 local head-dim columns
NDC = D // 128             # 16 contraction chunks
NSB = S // 512             # 4 s-blocks

F32 = mybir.dt.float32
BF16 = mybir.dt.bfloat16
AF = mybir.ActivationFunctionType
NPBF = ml_dtypes.bfloat16


def _mm(nc, out, lhsT, rhs, start, stop):
    nc.tensor.matmul(out, lhsT, rhs, start=start, stop=stop)


def _build():
    nc = bacc.Bacc("TRN2", target_bir_lowering=False, debug=False)

    # x and W arrive pre-packed host-side as [128, chunk, cols] so each DMA
    # moves 4KB-contiguous per-partition segments (vs 1KB unpacked).
    xT = nc.dram_tensor("xT", [NSB, 128, NDC, 512], BF16, kind="ExternalInput")
    wq = nc.dram_tensor("wq", [128, NDC, HL], BF16, kind="ExternalInput")
    wk = nc.dram_tensor("wk", [128, NDC, HL], BF16, kind="ExternalInput")
    wv = nc.dram_tensor("wv", [128, NDC, HL], BF16, kind="ExternalInput")
    wo = nc.dram_tensor("wo", [HL, D], BF16, kind="ExternalInput")
    cosT = nc.dram_tensor("cosT", [HD, S], BF16, kind="ExternalInput")
    sinT = nc.dram_tensor("sinT", [HD, S], BF16, kind="ExternalInput")
    pmatT = nc.dram_tensor("pmatT", [HD, HD], BF16, kind="ExternalInput")
    maskT = nc.dram_tensor("maskT", [128, 512], BF16, kind="ExternalInput")
    onesd = nc.dram_tensor("onesd", [128, 128], BF16, kind="ExternalInput")
    out = nc.dram_tensor("out", [S, D], BF16, kind="ExternalOutput")

    with TileContext(nc) as tc:
        with (
            tc.tile_pool(name="homes", bufs=1) as homes,
            tc.tile_pool(name="consts", bufs=1) as consts,
            tc.tile_pool(name="wpool", bufs=1) as wpool,
            tc.tile_pool(name="xtp", bufs=6) as xtp,
            tc.tile_pool(name="st1", bufs=3) as st1,
            tc.tile_pool(name="ptp", bufs=6) as ptp,
            tc.tile_pool(name="smp", bufs=2) as smp,
            tc.tile_pool(name="outp", bufs=2) as outp,
            tc.tile_pool(name="psA", bufs=4, space="PSUM") as psA,
            tc.tile_pool(name="psB", bufs=2, space="PSUM") as psB,
            tc.tile_pool(name="psC", bufs=2, space="PSUM") as psC,
        ):
            # persistent SBUF homes (bf16)
            qh = [[homes.tile([HD, 512], BF16, name=f"qh{h}_{sb}")
                   for sb in range(NSB)] for h in range(HPC)]
            kh = [[homes.tile([HD, 512], BF16, name=f"kh{h}_{sb}")
                   for sb in range(NSB)] for h in range(HPC)]
            vh = [homes.tile([128, HL], BF16, name=f"vh{kc}")
                  for kc in range(4 * NSB)]
            ch = [[homes.tile([HD, 512], BF16, name=f"ch{h}_{sb}")
                   for sb in range(NSB)] for h in range(HPC)]

            pmat_sb = consts.tile([HD, HD], BF16, name="pmat_sb")
            mask_sb = consts.tile([128, 512], BF16, name="mask_sb")
            ones_sb = consts.tile([128, 128], BF16, name="ones_sb")
            cos_sb = consts.tile([HD, S], BF16, name="cos_sb")
            sin_sb = consts.tile([HD, S], BF16, name="sin_sb")

            wo_sb = wpool.tile([128, HPC, D], BF16, name="wo_sb")

            # quarter loads: [128, 4, 512] tiles, 4KB contiguous per partition
            def load_w_quarter(w_d, q4, tag, eng):
                wt = wpool.tile([128, 4, HL], BF16, tag=f"{tag}{q4}", name="wt")
                eng.dma_start(out=wt[:], in_=w_d[:, q4 * 4:(q4 + 1) * 4, :])
                return wt

            def load_x_quarter(sb, q4):
                xt = xtp.tile([128, 4, 512], BF16, tag="xt", name="xt")
                nc.sync.dma_start(out=xt[:],
                                  in_=xT[sb, :, q4 * 4:(q4 + 1) * 4, :])
                return xt

            # ---------------- prologue DMAs (dual-queue) --------------------
            # sync: x quarters + cos + small consts + wv; scalar: wq + sin +
            # wk + wo
            xts = []
            wq_t, wk_t, wv_t = [], [], []
            for q4 in range(4):
                xts.append(load_x_quarter(0, q4))
                wq_t.append(load_w_quarter(wq, q4, "wq", nc.scalar))
                if q4 == 1:
                    nc.sync.dma_start(out=cos_sb[:], in_=cosT[:])
                    nc.scalar.dma_start(out=sin_sb[:], in_=sinT[:])
                if q4 == 2:
                    nc.sync.dma_start(out=pmat_sb[:], in_=pmatT[:])
                    nc.sync.dma_start(out=mask_sb[:], in_=maskT[:])
                    nc.sync.dma_start(out=ones_sb[:], in_=onesd[:])
            for q4 in range(4):
                wk_t.append(load_w_quarter(wk, q4, "wk", nc.scalar))
                wv_t.append(load_w_quarter(wv, q4, "wv", nc.sync))
            nc.scalar.dma_start(out=wo_sb[:],
                                in_=wo.rearrange("(r p) c -> p r c", p=128))

            def xchunk(xts_g, dc):
                return xts_g[dc // 4][:, dc % 4, :]

            # proj finishers, lagged one group behind emission
            finishers = deque()

            def emit_finisher():
                kind, args = finishers.popleft()
                if kind == "qk":
                    ps, dst, sl = args
                    qraw = st1.tile([128, 512], BF16, tag="qraw", name="qraw")
                    nc.scalar.activation(qraw[:], ps[:], AF.Copy)
                    rot = psB.tile([128, 512], F32, tag="b", name="rot")
                    _mm(nc, rot[:], pmat_sb[:], qraw[:], start=True, stop=True)
                    acos = st1.tile([128, 512], F32, tag="acos", name="acos")
                    nc.vector.tensor_mul(acos[:], ps[:], cos_sb[:, sl])
                    rsin = st1.tile([128, 512], F32, tag="rsin", name="rsin")
                    nc.vector.tensor_mul(rsin[:], rot[:], sin_sb[:, sl])
                    nc.vector.tensor_add(dst[:], rsin[:], acos[:])
                else:
                    ps, kc = args
                    nc.scalar.activation(vh[kc][:], ps[:], AF.Copy)

            # attention bookkeeping
            lagq = deque()    # (lps, pv, pt, vtc, ncols, first, last)
            fin = deque()     # (h, sb, lps, pv)
            lpv_done = set()

            def emit_lpv(job):
                lps, pv, pt, vtc, ncols, first, last = job
                _mm(nc, lps[:, 512 - ncols:], ones_sb[:], pt[:, :ncols],
                    start=first, stop=last)
                _mm(nc, pv[:, 512 - ncols:], vtc, pt[:, :ncols],
                    start=first, stop=last)
                if last:
                    lpv_done.add(id(lps))

            def emit_finalize(job):
                h, sb, lps, pv = job
                rcps = smp.tile([128, 512], F32, tag="rcps", name="rcps")
                rcp = smp.tile([128, 512], F32, tag="rcp", name="rcp")
                nc.vector.reciprocal_approx_accurate(rcp[:], lps[:], rcps[:])
                nc.vector.tensor_mul(ch[h][sb][:], pv[:], rcp[:])

            def pop_finalize():
                if fin and id(fin[0][2]) in lpv_done:
                    emit_finalize(fin.popleft())

            outproj_q = deque()   # (qc, db) pending output-projection groups
            osb_cur = {}          # qc -> [128, D] staging tile

            def emit_outproj():
                qc, db = outproj_q.popleft()
                sb = qc // 4
                ops = psA.tile([128, 512], F32, tag="a", name="ops")
                for h in range(HPC):
                    _mm(nc, ops[:],
                        ch[h][sb][:, (qc % 4) * 128:(qc % 4 + 1) * 128],
                        wo_sb[:, h, db * 512:(db + 1) * 512],
                        start=(h == 0), stop=(h == HPC - 1))
                if db == 0:
                    osb_cur[qc] = outp.tile([128, D], BF16, tag="osb", name="osb")
                nc.scalar.activation(
                    osb_cur[qc][:, db * 512:(db + 1) * 512], ops[:], AF.Copy)
                if db == D // 512 - 1:
                    nc.sync.dma_start(
                        out=out[qc * 128:(qc + 1) * 128, :],
                        in_=osb_cur.pop(qc)[:])

            def emit_qk_group(w_t, dst_tile, h, xts_g, sl_g):
                ps = psA.tile([128, 512], F32, tag="a", name="ps")
                for dc in range(NDC):
                    _mm(nc, ps[:],
                        w_t[dc // 4][:, dc % 4, h * HD:(h + 1) * HD],
                        xchunk(xts_g, dc),
                        start=(dc == 0), stop=(dc == NDC - 1))
                finishers.append(("qk", (ps, dst_tile, sl_g)))
                if len(finishers) > 1:
                    emit_finisher()

            def emit_v_group(sc, kc_out, xts_g):
                ps = psA.tile([128, 512], F32, tag="a", name="ps")
                for dc in range(NDC):
                    _mm(nc, ps[:],
                        xchunk(xts_g, dc)[:, sc * 128:(sc + 1) * 128],
                        wv_t[dc // 4][:, dc % 4, :],
                        start=(dc == 0), stop=(dc == NDC - 1))
                finishers.append(("v", (ps, kc_out)))
                if len(finishers) > 1:
                    emit_finisher()

            def outproj_drip(n):
                for _ in range(n):
                    if outproj_q:
                        emit_outproj()

            # ------------------------- main fused loop ----------------------
            for sb in range(NSB):
                sl = slice(sb * 512, (sb + 1) * 512)
                for h in range(HPC):
                    emit_qk_group(wq_t, qh[h][sb], h, xts, sl)
                    outproj_drip(1)
                for h in range(HPC):
                    emit_qk_group(wk_t, kh[h][sb], h, xts, sl)
                    outproj_drip(2)
                for sc in range(4):
                    emit_v_group(sc, 4 * sb + sc, xts)
                    outproj_drip(2)
                while finishers:
                    emit_finisher()
                while outproj_q:
                    emit_outproj()

                # x prefetch for the next s-block (streams during attention)
                if sb < NSB - 1:
                    xts = [load_x_quarter(sb + 1, q4) for q4 in range(4)]

                # ------- attention for q-block qb == sb, all local heads ----
                nk = 4 * sb + 4
                for h in range(HPC):
                    lps = psC.tile([128, 512], F32, tag="c", name="lps")
                    pv = psB.tile([128, 512], F32, tag="b", name="pv")
                    for kc in range(nk):
                        j = kc - 4 * sb
                        ncols = 512 if j < 0 else 512 - 128 * j
                        sps = psA.tile([128, 512], F32, tag="a", name="sps")
                        _mm(nc, sps[:, :ncols],
                            kh[h][kc // 4][:, (kc % 4) * 128:(kc % 4 + 1) * 128],
                            qh[h][sb][:, 512 - ncols:],
                            start=True, stop=True)
                        pt = ptp.tile([128, 512], BF16, tag="pt", name="pt")
                        nc.scalar.activation(pt[:, :ncols], sps[:, :ncols], AF.Exp)
                        if j >= 0:
                            nc.vector.tensor_mul(pt[:, :ncols], pt[:, :ncols],
                                                 mask_sb[:, :ncols])
                        lagq.append((lps, pv, pt,
                                     vh[kc][:, h * HD:(h + 1) * HD],
                                     ncols, kc == 0, kc == nk - 1))
                        while len(lagq) > 2:
                            emit_lpv(lagq.popleft())
                        if kc % 2 == 1:
                            pop_finalize()
                    fin.append((h, sb, lps, pv))
                while lagq:
                    emit_lpv(lagq.popleft())
                while fin:
                    pop_finalize()
                # queue this s-block's output projection
                for qc in range(4 * sb, 4 * sb + 4):
                    for db in range(D // 512):
                        outproj_q.append((qc, db))
            while outproj_q:
                emit_outproj()

    nc.compile()
    return nc


_NC_CACHE = None


def _get_nc():
    global _NC_CACHE
    if _NC_CACHE is None:
        _NC_CACHE = _build()
    return _NC_CACHE


def _host_tables():
    # Replicate reference RoPE tables in float32 arithmetic, transposed.
    inv_freq = np.float32(1.0) / np.power(
        np.float32(ROPE_THETA), np.arange(0, HD, 2).astype(np.float32) / np.float32(HD)
    )
    pos = np.arange(S, dtype=np.float32)
    freqs = pos[:, None] * inv_freq[None, :]
    angles = np.concatenate([freqs, freqs], axis=1)  # [S, HD]
    cos_t = np.ascontiguousarray(np.cos(angles).astype(np.float32).T).astype(NPBF)
    sin_t = np.ascontiguousarray(np.sin(angles).astype(np.float32).T).astype(NPBF)
    # rotate_half as a left-multiply matrix P: (P q)[2i] = -q[2i+1], [2i+1] = q[2i].
    # matmul computes lhsT.T @ rhs, so feed P.T.
    pmat = np.zeros((HD, HD), dtype=np.float32)
    for i in range(HD // 2):
        pmat[2 * i, 2 * i + 1] = -1.0
        pmat[2 * i + 1, 2 * i] = 1.0
    pmat_t = np.ascontiguousarray(pmat.T).astype(NPBF)
    mask = (np.arange(128)[:, None] <= np.arange(512)[None, :]).astype(NPBF)
    return cos_t, sin_t, pmat_t, mask


_ONES = np.ones((128, 128), dtype=NPBF)


def kernel(x, Wq, Wk, Wv, Wo):
    x = np.asarray(x, dtype=np.float32)
    Wq = np.asarray(Wq, dtype=np.float32)
    Wk = np.asarray(Wk, dtype=np.float32)
    Wv = np.asarray(Wv, dtype=np.float32)
    Wo = np.asarray(Wo, dtype=np.float32)

    results = _run_device(x, Wq, Wk, Wv, Wo)

    out = np.empty((B, S, D), dtype=np.float32)
    for b in range(B):
        acc = np.asarray(results[b * CORES_PER_BATCH]["out"], np.float32)
        for i in range(1, CORES_PER_BATCH):
            acc = acc + np.asarray(results[b * CORES_PER_BATCH + i]["out"],
                                   np.float32)
        out[b] = acc
    return out


def _pack_w(w):
    # [D, HL] -> [128, NDC, HL]: partition p holds rows {dc*128+p}, 4KB rows
    return np.ascontiguousarray(
        w.reshape(NDC, 128, HL).transpose(1, 0, 2)).astype(NPBF)


def _pack_x(xb):
    # x[b] [S, D] -> [NSB, 128, NDC, 512]: xT chunk layout per s-block
    xT = xb.T  # [D, S]
    return np.ascontiguousarray(
        xT.reshape(NDC, 128, NSB, 512).transpose(2, 1, 0, 3)).astype(NPBF)


def _make_in_maps(x, Wq, Wk, Wv, Wo):
    cos_t, sin_t, pmat_t, mask = _host_tables()
    scale = np.float32(1.0 / math.sqrt(HD))
    wq_scaled = (Wq * scale).astype(np.float32)
    xTb = [_pack_x(x[b]) for b in range(B)]
    in_maps = []
    for c in range(N_CORES):
        b = c // CORES_PER_BATCH
        g = c % CORES_PER_BATCH
        hs = slice(g * HL, (g + 1) * HL)
        in_maps.append({
            "xT": xTb[b],
            "wq": _pack_w(wq_scaled[:, hs]),
            "wk": _pack_w(Wk[:, hs]),
            "wv": _pack_w(Wv[:, hs]),
            "wo": np.ascontiguousarray(Wo[hs, :]).astype(NPBF),
            "cosT": cos_t,
            "sinT": sin_t,
            "pmatT": pmat_t,
            "maskT": mask,
            "onesd": _ONES,
        })
    return in_maps


def _run_device(x, Wq, Wk, Wv, Wo, trace=False):
    nc = _get_nc()
    in_maps = _make_in_maps(x, Wq, Wk, Wv, Wo)
    res = run_bass_kernel_spmd(nc, in_maps, core_ids=list(range(N_CORES)), trace=trace)
    if trace:
        return res
    return res.results


def run_traced(x, Wq, Wk, Wv, Wo):
    """Run with NTFF tracing; returns (full_output, BassKernelResults)."""
    res = _run_device(np.asarray(x, np.float32), np.asarray(Wq, np.float32),
                      np.asarray(Wk, np.float32), np.asarray(Wv, np.float32),
                      np.asarray(Wo, np.float32), trace=True)
    out = np.empty((B, S, D), dtype=np.float32)
    for b in range(B):
        acc = np.asarray(res.results[b * CORES_PER_BATCH]["out"], np.float32)
        for i in range(1, CORES_PER_BATCH):
            acc = acc + np.asarray(res.results[b * CORES_PER_BATCH + i]["out"],
                                   np.float32)
        out[b] = acc
    return out, res
